# revision 60
# baseline (speedup 1.0000x reference)
"""Trainium2 Bass kernel for batched scaled-dot-product attention.

Problem (reference math in fp32):
    q = queries @ Wq + bq          [B=4, N=4096, E=64]   (D_MODEL=768)
    k = keys    @ Wk + bk
    v = values  @ Wv + bv
    out = softmax(q k^T / sqrt(E)) @ v                    [B, N, 64]

Sharding: 8 cores, data-parallel over batch x query-half.  Core c handles
batch b=c//2, query rows [h*2048, (h+1)*2048) with h=c%2; it loads the full
keys/values for its batch (softmax needs every key).

v2 design (vs the fp32r v1 baseline at ~176us):
  * Everything on the input path is bf16 (host-cast): x DMA bytes halve to
    ~12MB/core and every matmul runs at 1 cycle/row at any PE p-state.
    Verified numerically: end-to-end rel err ~5.5e-3 vs the 2e-2 gate.
  * No q/k row-doubling: bf16 matmuls don't need a 128-deep contraction to
    hit full rate (the moving-row stream is the limit either way).
  * The 1/sqrt(E) scale is folded into Wq/bq on the host.
  * v is projected straight into natural [seq,64] layout ("va-direct"):
    per 128-row tile, 6 matmuls with the x_v^T chunk as the stationary
    operand.  No PE/DMA transposes anywhere in the main pipeline.  Two ones
    columns are appended (va width 66) so attention row-sums fall out of
    the av matmul; normalization happens on the HOST after gather.
  * Attention in S^T layout.  Query groups 0-2 stream inline with the k/v
    projection (per k-tile: 3 S matmuls, a paired exp on groups 0+1 plus a
    single exp on group 2, 3 av accumulations).  Group 3 runs as a second
    pass over resident kT/qT/va with kt-paired exps.  This 3+1 split is
    what fits 8 PSUM banks: S pool 2x[128,2,512] (4) + oT 3x[66,512] (3) +
    projection accumulator (1).
  * exp is the ACT-engine floor (~55us of pure column throughput); pairing
    two 512-col scores tiles per activation instruction halves the ~143ns
    per-instruction overhead.  ACT does nothing but exp.
  * Output is written as oT [66, 2048] fp32 (64 value rows + rowsum row);
    the host does out = (oT[:64]/oT[64]).T -- no device epilogue transpose.
"""

import numpy as np
import ml_dtypes

B, N, D, E = 4, 4096, 768, 64
NCORES = 8
HALF = N // 2          # query rows per core
CH = D // 128          # 6 feature chunks of the contraction dim
GROUP = 512            # query columns per group
QG = HALF // GROUP     # 4 query groups per core
KT = N // 128          # 32 key tiles
KG = N // GROUP        # 8 k/v projection groups
MA = E + 2             # va width: 64 values + 2 ones columns (rowsum)
SCALE = 0.125          # 1/sqrt(E), folded into Wq/bq on the host

_CACHE = {}


def _build():
    from contextlib import ExitStack

    import concourse.mybir as mybir
    import concourse.tile as tile
    from concourse import bacc

    f32 = mybir.dt.float32
    bf16 = mybir.dt.bfloat16
    EXP = mybir.ActivationFunctionType.Exp

    nc = bacc.Bacc(trn_type="TRN2")
    # x tensors are host-packed [128, CH, seq]: x_pre[p, c, s] = x[s, c*128+p]
    # so any seq-slice DMA moves long contiguous runs per partition.
    x_q = nc.dram_tensor("x_q", [128, CH, HALF], bf16, kind="ExternalInput")
    x_k = nc.dram_tensor("x_k", [128, CH, N], bf16, kind="ExternalInput")
    x_v = nc.dram_tensor("x_v", [128, CH, N], bf16, kind="ExternalInput")
    # weights packed as one tensor (fewer dma_starts: each costs ~1us of
    # descriptor generation on the SP sequencer at kernel start)
    w_all = nc.dram_tensor("w_all", [128, 3, CH, E], bf16, kind="ExternalInput")
    b_qk = nc.dram_tensor("b_qk", [E, 2], f32, kind="ExternalInput")
    b_v4 = nc.dram_tensor("b_v4", [128, 4, E], bf16, kind="ExternalInput")
    out = nc.dram_tensor("out", [MA, HALF], f32, kind="ExternalOutput")
    import os
    debug = bool(os.environ.get("KERNEL_DEBUG_DUMP"))
    if debug:
        dbg_qT = nc.dram_tensor("dbg_qT", [E, HALF], bf16, kind="ExternalOutput")
        dbg_kT = nc.dram_tensor("dbg_kT", [E, N], bf16, kind="ExternalOutput")
        dbg_va = nc.dram_tensor("dbg_va", [128, KT, MA], bf16, kind="ExternalOutput")

    with tile.TileContext(nc) as tc, ExitStack() as ctx:
        singles = ctx.enter_context(tc.tile_pool(name="singles", bufs=1))
        w_sb = singles.tile([128, 3, CH, E], bf16)
        bqk_sb = singles.tile([E, 2], f32)
        bv4_sb = singles.tile([128, 4, E], bf16)
        nc.sync.dma_start(out=w_sb, in_=w_all[:])
        nc.sync.dma_start(out=bqk_sb, in_=b_qk[:])
        nc.sync.dma_start(out=bv4_sb, in_=b_v4[:])
        wq_sb, wk_sb, wv_sb = w_sb[:, 0], w_sb[:, 1], w_sb[:, 2]
        bq_sb, bk_sb = bqk_sb[:, 0:1], bqk_sb[:, 1:2]

        qT = singles.tile([E, HALF], bf16)      # q^T/8 (+bias)
        kT = singles.tile([E, N], bf16)         # k^T
        va = singles.tile([128, KT, MA], bf16)  # v natural + two ones columns
        nc.vector.memset(va[:, :, E:MA], 1.0)

        xkvpool = ctx.enter_context(tc.tile_pool(name="xkv", bufs=6))
        xgpool = ctx.enter_context(tc.tile_pool(name="xg", bufs=5))
        xpools = {"xkv": xkvpool, "xg": xgpool}
        pt01 = ctx.enter_context(tc.tile_pool(name="pt01", bufs=7))
        eppool = ctx.enter_context(tc.tile_pool(name="ep", bufs=2))
        spool = ctx.enter_context(tc.tile_pool(name="s", bufs=2, space="PSUM"))
        opool = ctx.enter_context(tc.tile_pool(name="o", bufs=3, space="PSUM"))
        pjpool = ctx.enter_context(tc.tile_pool(name="pj", bufs=1, space="PSUM"))

        def load_x(x_dr, s0, s1, tag="xkv"):
            xt = xpools[tag].tile(
                [128, CH, s1 - s0], bf16, tag=tag, name="xt")
            nc.sync.dma_start(out=xt, in_=x_dr[:, :, s0:s1])
            return xt

        def proj_qk(xt, off, w, b_sb, dst, g, pool=None, tag="pj"):
            pool = pool or pjpool
            ps = pool.tile([128, GROUP], f32, tag=tag, name="ps")
            for c in range(CH):
                nc.tensor.matmul(
                    ps[:E], lhsT=w[:, c, :], rhs=xt[:, c, off:off + GROUP],
                    start=(c == 0), stop=(c == CH - 1))
            nc.vector.tensor_scalar_add(
                dst[:, g * GROUP:(g + 1) * GROUP], ps[:E], b_sb)

        def proj_va(xt, off, g8):
            # va-direct: project straight into natural [seq,64] layout with
            # the x_v^T chunk as the stationary operand (PE transposes and
            # XBAR DMA transposes both measure far slower on hardware).
            # c must be the inner loop: a start=True resets the whole PSUM
            # bank's accumulation state, so groups cannot interleave.
            vj = pjpool.tile([128, 4, E], f32, tag="pj", name="vj")
            for t in range(4):
                for c in range(CH):
                    nc.tensor.matmul(
                        vj[:, t, :],
                        lhsT=xt[:, c, off + t * 128:off + (t + 1) * 128],
                        rhs=wv_sb[:, c, :], start=(c == 0), stop=(c == CH - 1),
                        skip_group_check=True)
            nc.vector.tensor_add(va[:, g8 * 4:(g8 + 1) * 4, :E], vj, bv4_sb)

        def s_step(kt, g, out_ap):
            nc.tensor.matmul(
                out_ap,
                lhsT=kT[:, kt * 128:(kt + 1) * 128],
                rhs=qT[:, g * GROUP:(g + 1) * GROUP],
                start=True, stop=True, skip_group_check=True)

        def av_step(oT_g, kt, pt_ap, first, last):
            nc.tensor.matmul(
                oT_g, lhsT=va[:, kt, :], rhs=pt_ap,
                start=first, stop=last, skip_group_check=True)

        def epilogue(g, oT_g):
            o_sb = eppool.tile([MA, GROUP], f32, tag="ep", name="o_sb")
            nc.vector.tensor_copy(o_sb, oT_g)
            nc.sync.dma_start(out=out[:, g * GROUP:(g + 1) * GROUP], in_=o_sb)

        # ---- prologue: small q0 DMA first, then k/v tile 0, then q1/q2 ----
        xq0 = load_x(x_q, 0, GROUP, tag="xg")
        xk_t = {0: load_x(x_k, 0, 1024)}
        xv_t = {0: load_x(x_v, 0, 1024)}
        xq12 = load_x(x_q, GROUP, 3 * GROUP)
        proj_qk(xq0, 0, wq_sb, bq_sb, qT, 0)

        oT = [opool.tile([MA, GROUP], f32, tag="o", name=f"oT{g}")
              for g in range(3)]
        # pend holds deferred av accumulations: (group_idx, kt, pT ap).
        # They are flushed one attention step later so the in-order PE never
        # waits on the exp it just requested.
        pend = []

        def flush_av():
            while pend:
                g, kt, ap = pend.pop(0)  # FIFO: the kt==0 start goes first
                av_step(oT[g], kt, ap, kt == 0, kt == KT - 1)

        def attention_kt(kt):
            X = spool.tile([128, 2, GROUP], f32, tag="s", name="X")
            s_step(kt, 0, X[:, 0, :])
            s_step(kt, 1, X[:, 1, :])
            p01 = pt01.tile([128, 2, GROUP], bf16, tag="pt", name="p01")
            nc.scalar.activation(p01, X, EXP)
            Y = spool.tile([128, 2, GROUP], f32, tag="s", name="Y")
            s_step(kt, 2, Y[:, 0, :])
            p2 = pt01.tile([128, 2, GROUP], bf16, tag="pt", name="p2")
            nc.scalar.activation(p2[:, 0, :], Y[:, 0, :], EXP)
            flush_av()
            pend.append((0, kt, p01[:, 0, :]))
            pend.append((1, kt, p01[:, 1, :]))
            pend.append((2, kt, p2[:, 0, :]))

        def attention_pair(kt):
            attention_kt(kt)
            attention_kt(kt + 1)

        proj_qk(xk_t[0], 0, wk_sb, bk_sb, kT, 0)
        proj_qk(xq12, 0, wq_sb, bq_sb, qT, 1)
        proj_qk(xq12, GROUP, wq_sb, bq_sb, qT, 2)
        proj_va(xv_t[0], 0, 0)
        xk_t[1] = load_x(x_k, 1024, 2048)
        xv_t[1] = load_x(x_v, 1024, 2048)
        for g8 in range(1, KG):
            d, half = divmod(g8, 2)
            if half == 0 and d + 1 < KG // 2:
                xk_t[d + 1] = load_x(x_k, (d + 1) * 1024, (d + 2) * 1024)
                xv_t[d + 1] = load_x(x_v, (d + 1) * 1024, (d + 2) * 1024)
            base = 4 * (g8 - 1)
            proj_qk(xk_t[d], half * GROUP, wk_sb, bk_sb, kT, g8)
            attention_pair(base)
            proj_va(xv_t[d], half * GROUP, g8)
            if g8 == 1:
                # group-3 q projection, off the critical path
                xq3 = load_x(x_q, 3 * GROUP, HALF, tag="xg")
                proj_qk(xq3, 0, wq_sb, bq_sb, qT, 3)
            attention_pair(base + 2)
        attention_pair(4 * (KG - 1))
        attention_pair(4 * (KG - 1) + 2)
        flush_av()
        for g in range(3):
            epilogue(g, oT[g])

        # ---- pass 2: group 3 over resident kT/qT/va, kt-paired exps ----
        oT3 = opool.tile([MA, GROUP], f32, tag="o", name="oT3")
        pend3 = []
        for kp in range(KT // 2):
            Z = spool.tile([128, 2, GROUP], f32, tag="s", name="Z")
            s_step(2 * kp, 3, Z[:, 0, :])
            s_step(2 * kp + 1, 3, Z[:, 1, :])
            p3 = pt01.tile([128, 2, GROUP], bf16, tag="pt", name="p3")
            nc.scalar.activation(p3, Z, EXP)
            while pend3:
                pkp, pp = pend3.pop()
                av_step(oT3, 2 * pkp, pp[:, 0, :], pkp == 0, False)
                av_step(oT3, 2 * pkp + 1, pp[:, 1, :], False,
                        pkp == KT // 2 - 1)
            pend3.append((kp, p3))
        while pend3:
            pkp, pp = pend3.pop()
            av_step(oT3, 2 * pkp, pp[:, 0, :], pkp == 0, False)
            av_step(oT3, 2 * pkp + 1, pp[:, 1, :], False, pkp == KT // 2 - 1)
        epilogue(3, oT3)

        if debug:
            nc.sync.dma_start(out=dbg_qT[:], in_=qT)
            nc.sync.dma_start(out=dbg_kT[:], in_=kT)
            nc.sync.dma_start(out=dbg_va[:], in_=va)

    nc.finalize()
    return nc


def get_nc():
    if "nc" not in _CACHE:
        _CACHE["nc"] = _build()
    return _CACHE["nc"]


def make_in_maps(queries, keys, values, Wq, bq, Wk, bk, Wv, bv):
    bf = ml_dtypes.bfloat16

    def xt(a):  # [seq, D] fp32 -> packed bf16 [128, CH, seq]
        at = np.asarray(a, dtype=np.float32).T  # [D, seq]
        return np.ascontiguousarray(
            at.reshape(CH, 128, at.shape[1]).transpose(1, 0, 2).astype(bf))

    def wpack(w, scale=1.0):  # [D, E] -> [128, CH, E] bf16
        w = np.asarray(w, dtype=np.float32) * scale
        return np.ascontiguousarray(
            w.reshape(CH, 128, E).transpose(1, 0, 2).astype(bf))

    queries = np.asarray(queries, dtype=np.float32)
    keys = np.asarray(keys, dtype=np.float32)
    values = np.asarray(values, dtype=np.float32)
    shared = {
        "w_all": np.ascontiguousarray(np.stack(
            [wpack(Wq, SCALE), wpack(Wk), wpack(Wv)], axis=1)),
        "b_qk": np.ascontiguousarray(np.stack(
            [np.asarray(bq, np.float32) * SCALE,
             np.asarray(bk, np.float32)], axis=1)),
        "b_v4": np.ascontiguousarray(np.broadcast_to(
            np.asarray(bv, np.float32).astype(bf), (128, 4, E))),
    }
    in_maps = []
    for c in range(NCORES):
        b, h = divmod(c, 2)
        in_maps.append({
            "x_q": xt(queries[b, h * HALF:(h + 1) * HALF, :]),
            "x_k": xt(keys[b]),
            "x_v": xt(values[b]),
            **shared,
        })
    return in_maps


def run(trace=False, **inputs):
    from concourse.bass_utils import run_bass_kernel_spmd

    nc = get_nc()
    in_maps = make_in_maps(**inputs)
    res = run_bass_kernel_spmd(
        nc, in_maps, core_ids=list(range(NCORES)), trace=trace)
    full = np.empty((B, N, E), dtype=np.float32)
    for c in range(NCORES):
        b, h = divmod(c, 2)
        o = np.asarray(res.results[c]["out"], dtype=np.float32)  # [66, 2048]
        full[b, h * HALF:(h + 1) * HALF, :] = (o[:E] / o[E:E + 1]).T
    return full, res


def kernel(**inputs):
    full, _ = run(trace=False, **inputs)
    return full


# revision 62
# speedup vs baseline: 1.2121x; 1.2121x over previous
"""Trainium2 Bass kernel for batched scaled-dot-product attention.

Problem (reference math in fp32):
    q = queries @ Wq + bq          [B=4, N=4096, E=64]   (D_MODEL=768)
    k = keys    @ Wk + bk
    v = values  @ Wv + bv
    out = softmax(q k^T / sqrt(E)) @ v                    [B, N, 64]

Sharding: 8 cores, data-parallel over batch x query-half.  Core c handles
batch b=c//2, query rows [h*2048, (h+1)*2048) with h=c%2; it loads the full
keys/values for its batch (softmax needs every key).

v2 design (vs the fp32r v1 baseline at ~176us):
  * Everything on the input path is bf16 (host-cast): x DMA bytes halve to
    ~12MB/core and every matmul runs at 1 cycle/row at any PE p-state.
    Verified numerically: end-to-end rel err ~5.5e-3 vs the 2e-2 gate.
  * No q/k row-doubling: bf16 matmuls don't need a 128-deep contraction to
    hit full rate (the moving-row stream is the limit either way).
  * The 1/sqrt(E) scale is folded into Wq/bq on the host.
  * v is projected straight into natural [seq,64] layout ("va-direct"):
    per 128-row tile, 6 matmuls with the x_v^T chunk as the stationary
    operand.  No PE/DMA transposes anywhere in the main pipeline.  Two ones
    columns are appended (va width 66) so attention row-sums fall out of
    the av matmul; normalization happens on the HOST after gather.
  * Attention in S^T layout.  Query groups 0-2 stream inline with the k/v
    projection (per k-tile: 3 S matmuls, a paired exp on groups 0+1 plus a
    single exp on group 2, 3 av accumulations).  Group 3 runs as a second
    pass over resident kT/qT/va with kt-paired exps.  This 3+1 split is
    what fits 8 PSUM banks: S pool 2x[128,2,512] (4) + oT 3x[66,512] (3) +
    projection accumulator (1).
  * exp is the ACT-engine floor (~55us of pure column throughput); pairing
    two 512-col scores tiles per activation instruction halves the ~143ns
    per-instruction overhead.  ACT does nothing but exp.
  * Output is written as oT [66, 2048] fp32 (64 value rows + rowsum row);
    the host does out = (oT[:64]/oT[64]).T -- no device epilogue transpose.
"""

import numpy as np
import ml_dtypes

B, N, D, E = 4, 4096, 768, 64
NCORES = 8
HALF = N // 2          # query rows per core
CH = D // 128          # 6 feature chunks of the contraction dim
GROUP = 512            # query columns per group
QG = HALF // GROUP     # 4 query groups per core
KT = N // 128          # 32 key tiles
KG = N // GROUP        # 8 k/v projection groups
MA = E + 2             # va width: 64 values + 2 ones columns (rowsum)
SCALE = 0.125          # 1/sqrt(E), folded into Wq/bq on the host

_CACHE = {}


def _build():
    from contextlib import ExitStack

    import concourse.mybir as mybir
    import concourse.tile as tile
    from concourse import bacc

    f32 = mybir.dt.float32
    bf16 = mybir.dt.bfloat16
    EXP = mybir.ActivationFunctionType.Exp

    nc = bacc.Bacc(trn_type="TRN2")
    # x tensors are host-packed [128, CH, seq]: x_pre[p, c, s] = x[s, c*128+p]
    # so any seq-slice DMA moves long contiguous runs per partition.
    x_q = nc.dram_tensor("x_q", [128, CH, HALF], bf16, kind="ExternalInput")
    x_k = nc.dram_tensor("x_k", [128, CH, N], bf16, kind="ExternalInput")
    x_v = nc.dram_tensor("x_v", [128, CH, N], bf16, kind="ExternalInput")
    # weights packed as one tensor (fewer dma_starts: each costs ~1us of
    # descriptor generation on the SP sequencer at kernel start)
    w_all = nc.dram_tensor("w_all", [128, 3, CH, E], bf16, kind="ExternalInput")
    b_qk = nc.dram_tensor("b_qk", [E, 2], f32, kind="ExternalInput")
    b_v4 = nc.dram_tensor("b_v4", [128, 4, E], bf16, kind="ExternalInput")
    out = nc.dram_tensor("out", [MA, HALF], f32, kind="ExternalOutput")
    import os
    debug = bool(os.environ.get("KERNEL_DEBUG_DUMP"))
    if debug:
        dbg_qT = nc.dram_tensor("dbg_qT", [E, HALF], bf16, kind="ExternalOutput")
        dbg_kT = nc.dram_tensor("dbg_kT", [E, N], bf16, kind="ExternalOutput")
        dbg_va = nc.dram_tensor("dbg_va", [128, KT, MA], bf16, kind="ExternalOutput")

    with tile.TileContext(nc) as tc, ExitStack() as ctx:
        singles = ctx.enter_context(tc.tile_pool(name="singles", bufs=1))
        w_sb = singles.tile([128, 3, CH, E], bf16)
        bqk_sb = singles.tile([E, 2], f32)
        bv4_sb = singles.tile([128, 4, E], bf16)
        nc.sync.dma_start(out=w_sb, in_=w_all[:])
        nc.sync.dma_start(out=bqk_sb, in_=b_qk[:])
        nc.sync.dma_start(out=bv4_sb, in_=b_v4[:])
        wq_sb, wk_sb, wv_sb = w_sb[:, 0], w_sb[:, 1], w_sb[:, 2]
        bq_sb, bk_sb = bqk_sb[:, 0:1], bqk_sb[:, 1:2]

        qT = singles.tile([E, HALF], bf16)      # q^T/8 (+bias)
        kT = singles.tile([E, N], bf16)         # k^T
        va = singles.tile([128, KT, MA], bf16)  # v natural + two ones columns
        nc.vector.memset(va[:, :, E:MA], 1.0)

        xkvpool = ctx.enter_context(tc.tile_pool(name="xkv", bufs=6))
        xgpool = ctx.enter_context(tc.tile_pool(name="xg", bufs=5))
        xpools = {"xkv": xkvpool, "xg": xgpool}
        pt01 = ctx.enter_context(tc.tile_pool(name="pt01", bufs=7))
        eppool = ctx.enter_context(tc.tile_pool(name="ep", bufs=2))
        spool = ctx.enter_context(tc.tile_pool(name="s", bufs=2, space="PSUM"))
        opool = ctx.enter_context(tc.tile_pool(name="o", bufs=3, space="PSUM"))
        pjpool = ctx.enter_context(tc.tile_pool(name="pj", bufs=1, space="PSUM"))

        def load_x(x_dr, s0, s1, tag="xkv"):
            xt = xpools[tag].tile(
                [128, CH, s1 - s0], bf16, tag=tag, name="xt")
            nc.sync.dma_start(out=xt, in_=x_dr[:, :, s0:s1])
            return xt

        def proj_qk(xt, off, w, b_sb, dst, g, pool=None, tag="pj"):
            pool = pool or pjpool
            ps = pool.tile([128, GROUP], f32, tag=tag, name="ps")
            for c in range(CH):
                nc.tensor.matmul(
                    ps[:E], lhsT=w[:, c, :], rhs=xt[:, c, off:off + GROUP],
                    start=(c == 0), stop=(c == CH - 1))
            nc.vector.tensor_scalar_add(
                dst[:, g * GROUP:(g + 1) * GROUP], ps[:E], b_sb)

        def proj_va(xt, off, g8):
            # va-direct: project straight into natural [seq,64] layout with
            # the x_v^T chunk as the stationary operand (PE transposes and
            # XBAR DMA transposes both measure far slower on hardware).
            # c must be the inner loop: a start=True resets the whole PSUM
            # bank's accumulation state, so groups cannot interleave.
            vj = pjpool.tile([128, 4, E], f32, tag="pj", name="vj")
            for t in range(4):
                for c in range(CH):
                    nc.tensor.matmul(
                        vj[:, t, :],
                        lhsT=xt[:, c, off + t * 128:off + (t + 1) * 128],
                        rhs=wv_sb[:, c, :], start=(c == 0), stop=(c == CH - 1),
                        skip_group_check=True)
            nc.vector.tensor_add(va[:, g8 * 4:(g8 + 1) * 4, :E], vj, bv4_sb)

        def s_step(kt, g, out_ap):
            nc.tensor.matmul(
                out_ap,
                lhsT=kT[:, kt * 128:(kt + 1) * 128],
                rhs=qT[:, g * GROUP:(g + 1) * GROUP],
                start=True, stop=True, skip_group_check=True)

        def av_step(oT_g, kt, pt_ap, first, last):
            nc.tensor.matmul(
                oT_g, lhsT=va[:, kt, :], rhs=pt_ap,
                start=first, stop=last, skip_group_check=True)

        def epilogue(g, oT_g):
            o_sb = eppool.tile([MA, GROUP], f32, tag="ep", name="o_sb")
            nc.vector.tensor_copy(o_sb, oT_g)
            nc.sync.dma_start(out=out[:, g * GROUP:(g + 1) * GROUP], in_=o_sb)

        # ---- prologue: small q0 DMA first, then k/v tile 0, then q1/q2 ----
        xq0 = load_x(x_q, 0, GROUP, tag="xg")
        xk_t = {0: load_x(x_k, 0, 1024)}
        xv_t = {0: load_x(x_v, 0, 1024)}
        xq12 = load_x(x_q, GROUP, 3 * GROUP)
        # q0 goes through the pj bank; q1/q2 rotate through the (still
        # unallocated) oT bank slots so the prologue projections pipeline
        # instead of serializing on the single pj bank.  The oT tiles are
        # allocated only afterwards, so there is no lifetime overlap.
        proj_qk(xq0, 0, wq_sb, bq_sb, qT, 0)

        # pend holds deferred av accumulations: (group_idx, kt, pT ap).
        # They are flushed one attention step later so the in-order PE never
        # waits on the exp it just requested.
        pend = []

        def flush_av():
            while pend:
                g, kt, ap = pend.pop(0)  # FIFO: the kt==0 start goes first
                av_step(oT[g], kt, ap, kt == 0, kt == KT - 1)

        def attention_kt(kt):
            X = spool.tile([128, 2, GROUP], f32, tag="s", name="X")
            s_step(kt, 0, X[:, 0, :])
            s_step(kt, 1, X[:, 1, :])
            p01 = pt01.tile([128, 2, GROUP], bf16, tag="pt", name="p01")
            nc.scalar.activation(p01, X, EXP)
            Y = spool.tile([128, 2, GROUP], f32, tag="s", name="Y")
            s_step(kt, 2, Y[:, 0, :])
            p2 = pt01.tile([128, 2, GROUP], bf16, tag="pt", name="p2")
            nc.scalar.activation(p2[:, 0, :], Y[:, 0, :], EXP)
            flush_av()
            pend.append((0, kt, p01[:, 0, :]))
            pend.append((1, kt, p01[:, 1, :]))
            pend.append((2, kt, p2[:, 0, :]))

        def attention_pair(kt):
            attention_kt(kt)
            attention_kt(kt + 1)

        proj_qk(xk_t[0], 0, wk_sb, bk_sb, kT, 0)
        proj_qk(xq12, 0, wq_sb, bq_sb, qT, 1, pool=opool, tag="o")
        proj_qk(xq12, GROUP, wq_sb, bq_sb, qT, 2, pool=opool, tag="o")
        proj_va(xv_t[0], 0, 0)
        oT = [opool.tile([MA, GROUP], f32, tag="o", name=f"oT{g}")
              for g in range(3)]
        xk_t[1] = load_x(x_k, 1024, 2048)
        xv_t[1] = load_x(x_v, 1024, 2048)
        for g8 in range(1, KG):
            d, half = divmod(g8, 2)
            if half == 0 and d + 1 < KG // 2:
                xk_t[d + 1] = load_x(x_k, (d + 1) * 1024, (d + 2) * 1024)
                xv_t[d + 1] = load_x(x_v, (d + 1) * 1024, (d + 2) * 1024)
            base = 4 * (g8 - 1)
            proj_qk(xk_t[d], half * GROUP, wk_sb, bk_sb, kT, g8)
            attention_pair(base)
            proj_va(xv_t[d], half * GROUP, g8)
            if g8 == 1:
                # group-3 q projection, off the critical path
                xq3 = load_x(x_q, 3 * GROUP, HALF, tag="xg")
                proj_qk(xq3, 0, wq_sb, bq_sb, qT, 3)
            attention_pair(base + 2)
        attention_pair(4 * (KG - 1))
        attention_pair(4 * (KG - 1) + 2)
        flush_av()
        for g in range(3):
            epilogue(g, oT[g])

        # ---- pass 2: group 3 over resident kT/qT/va, kt-paired exps ----
        oT3 = opool.tile([MA, GROUP], f32, tag="o", name="oT3")
        pend3 = []
        for kp in range(KT // 2):
            Z = spool.tile([128, 2, GROUP], f32, tag="s", name="Z")
            s_step(2 * kp, 3, Z[:, 0, :])
            s_step(2 * kp + 1, 3, Z[:, 1, :])
            p3 = pt01.tile([128, 2, GROUP], bf16, tag="pt", name="p3")
            nc.scalar.activation(p3, Z, EXP)
            while pend3:
                pkp, pp = pend3.pop()
                av_step(oT3, 2 * pkp, pp[:, 0, :], pkp == 0, False)
                av_step(oT3, 2 * pkp + 1, pp[:, 1, :], False,
                        pkp == KT // 2 - 1)
            pend3.append((kp, p3))
        while pend3:
            pkp, pp = pend3.pop()
            av_step(oT3, 2 * pkp, pp[:, 0, :], pkp == 0, False)
            av_step(oT3, 2 * pkp + 1, pp[:, 1, :], False, pkp == KT // 2 - 1)
        epilogue(3, oT3)

        if debug:
            nc.sync.dma_start(out=dbg_qT[:], in_=qT)
            nc.sync.dma_start(out=dbg_kT[:], in_=kT)
            nc.sync.dma_start(out=dbg_va[:], in_=va)

    nc.finalize()
    return nc


def get_nc():
    if "nc" not in _CACHE:
        _CACHE["nc"] = _build()
    return _CACHE["nc"]


def make_in_maps(queries, keys, values, Wq, bq, Wk, bk, Wv, bv):
    bf = ml_dtypes.bfloat16

    def xt(a):  # [seq, D] fp32 -> packed bf16 [128, CH, seq]
        at = np.asarray(a, dtype=np.float32).T  # [D, seq]
        return np.ascontiguousarray(
            at.reshape(CH, 128, at.shape[1]).transpose(1, 0, 2).astype(bf))

    def wpack(w, scale=1.0):  # [D, E] -> [128, CH, E] bf16
        w = np.asarray(w, dtype=np.float32) * scale
        return np.ascontiguousarray(
            w.reshape(CH, 128, E).transpose(1, 0, 2).astype(bf))

    queries = np.asarray(queries, dtype=np.float32)
    keys = np.asarray(keys, dtype=np.float32)
    values = np.asarray(values, dtype=np.float32)
    shared = {
        "w_all": np.ascontiguousarray(np.stack(
            [wpack(Wq, SCALE), wpack(Wk), wpack(Wv)], axis=1)),
        "b_qk": np.ascontiguousarray(np.stack(
            [np.asarray(bq, np.float32) * SCALE,
             np.asarray(bk, np.float32)], axis=1)),
        "b_v4": np.ascontiguousarray(np.broadcast_to(
            np.asarray(bv, np.float32).astype(bf), (128, 4, E))),
    }
    in_maps = []
    for c in range(NCORES):
        b, h = divmod(c, 2)
        in_maps.append({
            "x_q": xt(queries[b, h * HALF:(h + 1) * HALF, :]),
            "x_k": xt(keys[b]),
            "x_v": xt(values[b]),
            **shared,
        })
    return in_maps


def run(trace=False, **inputs):
    from concourse.bass_utils import run_bass_kernel_spmd

    nc = get_nc()
    in_maps = make_in_maps(**inputs)
    res = run_bass_kernel_spmd(
        nc, in_maps, core_ids=list(range(NCORES)), trace=trace)
    full = np.empty((B, N, E), dtype=np.float32)
    for c in range(NCORES):
        b, h = divmod(c, 2)
        o = np.asarray(res.results[c]["out"], dtype=np.float32)  # [66, 2048]
        full[b, h * HALF:(h + 1) * HALF, :] = (o[:E] / o[E:E + 1]).T
    return full, res


def kernel(**inputs):
    full, _ = run(trace=False, **inputs)
    return full
